# revision 1
# baseline (speedup 1.0000x reference)
"""Causal self-attention (B=1, S=4096, D=1024, 16 heads) on 8 trn2 NeuronCores.

Sharding: tensor-parallel over heads (2 heads per core). Each core computes
qkv projection for its head pair, causal attention, and a partial output
projection; the host sums the 8 partials and adds b_out.

Device kernel (per core, all matmuls in float32r, fp32 PSUM accumulation):
  Emission interleaves projection / output-projection work INTO the
  attention j-loop at single-matmul granularity (~213ns quanta) so the
  in-order PE stream has fill during the QK->exp->PV dependency gaps (exp
  on ACT is the j-loop rate limiter at ~1038ns/tile vs PE's ~854ns), and
  each PV is deferred one j-tile so exp latency never exposes on PE.
  - projections (per 512-seq chunk, split into per-m quanta): qT/kT/vT =
    w_shard.T @ xT; bias added on the DVE copy out of PSUM. Head B's q/k
    rows are relocated to partitions 0-63 via SBUF->SBUF DMA (SWDGE). V
    (natural layout) is built by PE-transposing vT k-tiles.
  - attention (per 512-wide q chunk): scores^T = K_tile.T @ Q per head
    (K=64), exp on ACT (scale=1/8 fused; scores are bounded so no
    max-subtraction is needed), causal-mask multiply on diagonal tiles, PV
    with a leading ones-column ([1|0pad|V], M=128, V dims at columns
    64-127) so the softmax denominator lands on PSUM partition 0 — the
    canonical partition_broadcast source on real HW (no relocation hop) —
    and ctx rows sit 64-aligned; reciprocal -> partition-broadcast ->
    normalize, with head B's mul writing ctxn[64:128] directly (aligned)
    and only head A DMA-relocated to partitions 0-63 for the K=128
    out-projection.
  - diagonal j-tiles are widened to a >=256 moving dim (fp32r matmul drops
    to 4 cycles/row below 256); the extra columns are fully masked.
  - x loads (HWDGE on SP) prefetch 2 chunks ahead; the first projection's
    dependencies (biases, w m=0, x chunk 0) are loaded first.
  - PSUM: scores 2x2 banks, ctxA/ctxB 1+1, proj accum + transposes 1,
    out-proj 1; the tail out-projection alternates into the then-idle
    scores ring so copies overlap matmuls, and the last j-loop (which has
    no projection fill) receives all of the previous chunk's deferred
    out-projection backlog.
"""
import sys

sys.path.insert(0, "/opt/trn_rl_repo")

from contextlib import ExitStack

import numpy as np

import concourse.tile as tile
from concourse import bacc, mybir
from concourse.alu_op_type import AluOpType
from concourse.masks import make_identity
from concourse.bass_utils import run_bass_kernel_spmd

D = 1024
N_CORES = 8
F32 = mybir.dt.float32
F32R = mybir.dt.float32r
AF = mybir.ActivationFunctionType

QC = 512  # q-chunk width
KT = 128  # k-tile width


def build_program(S: int = 4096, repeat: int = 1):
    nqc = S // QC

    nc = bacc.Bacc(None)
    xT = nc.declare_dram_parameter("xT", [D, S], F32R, isOutput=False)
    w_sh = nc.declare_dram_parameter("w_sh", [D, 384], F32R, isOutput=False)
    b_sh = nc.declare_dram_parameter("b_sh", [384], F32, isOutput=False)
    w_o = nc.declare_dram_parameter("w_o", [128, D], F32R, isOutput=False)
    outp = nc.declare_dram_parameter("outp", [S, D], F32, isOutput=True)

    with tile.TileContext(nc) as tc, ExitStack() as ctx:
        consts = ctx.enter_context(tc.tile_pool(name="consts", bufs=1))
        big = ctx.enter_context(tc.tile_pool(name="big", bufs=1))
        xpool = ctx.enter_context(tc.tile_pool(name="xp", bufs=2))
        vtpool = ctx.enter_context(tc.tile_pool(name="vt", bufs=2))
        stpool = ctx.enter_context(tc.tile_pool(name="st", bufs=2))
        apool = ctx.enter_context(tc.tile_pool(name="at", bufs=5))
        npool = ctx.enter_context(tc.tile_pool(name="nrm", bufs=1))
        opool = ctx.enter_context(tc.tile_pool(name="ot", bufs=2))
        psS = ctx.enter_context(tc.tile_pool(name="psS", bufs=2, space="PSUM"))
        psCA = ctx.enter_context(tc.tile_pool(name="psCA", bufs=1, space="PSUM"))
        psCB = ctx.enter_context(tc.tile_pool(name="psCB", bufs=1, space="PSUM"))
        psP = ctx.enter_context(tc.tile_pool(name="psP", bufs=1, space="PSUM"))
        psQ = ctx.enter_context(tc.tile_pool(name="psQ", bufs=1, space="PSUM"))

        # ---- constants
        ident_f = consts.tile([128, 128], F32)
        make_identity(nc, ident_f[:])
        ident = consts.tile([128, 128], F32R)
        nc.vector.tensor_copy(ident[:], ident_f[:])

        ones_f = consts.tile([128, 8], F32)
        nc.gpsimd.memset(ones_f[:], 1.0)
        # memset cannot encode an fp32r fill value (ISA check); zero the V
        # tiles via an F32 staging tile + dtype-converting copy instead
        zeros_f = consts.tile([128, 4, 256], F32)
        nc.gpsimd.memset(zeros_f[:], 0.0)

        def emit_xload(n, fine=False):
            # fine=True splits into t-pair DMAs so the first projection's
            # t=0 matmul can start after ~1/4 of the transfer
            xt = xpool.tile([128, 8, QC], F32R, tag="xt", name="xt")
            src = xT.rearrange("(t p) s -> p t s", p=128)
            step = 2 if fine else 4
            for t0 in range(0, 8, step):
                nc.sync.dma_start(
                    xt[:, t0:t0 + step, :],
                    src[:, t0:t0 + step, n * QC:(n + 1) * QC],
                )
            return xt

        # load order: first projection's deps (biases, w_sb m=0 slice, x
        # chunk 0) go first so proj(0) starts ~5us earlier
        w_sb = consts.tile([128, 8, 384], F32R)
        biases = consts.tile([128, 3], F32)
        nc.sync.dma_start(biases[:], b_sh.rearrange("(m p) -> p m", p=128))
        w_src = w_sh.rearrange("(t p) m -> p t m", p=128)
        nc.sync.dma_start(w_sb[:, :, 0:128], w_src[:, :, 0:128])
        xts_pre = {0: emit_xload(0, fine=True)}
        for m in range(1, 3):
            nc.sync.dma_start(
                w_sb[:, :, m * 128:(m + 1) * 128], w_src[:, :, m * 128:(m + 1) * 128]
            )
        xts_pre[1] = emit_xload(1)
        w_o_sb = consts.tile([128, D], F32R)
        nc.sync.dma_start(w_o_sb[:], w_o[:])

        # per-chunk projection tiles (separate tags so attention on chunk c
        # only depends on projections of chunks <= c)
        qk_t = [
            big.tile([64, 2, 2, QC], F32R, tag=f"qk{n}", name=f"qk{n}")
            for n in range(nqc)
        ]
        v_t = [
            big.tile([128, 4, 256], F32R, tag=f"v{n}", name=f"v{n}")
            for n in range(nqc)
        ]
        # ones-column FIRST, V dims at columns 64-127 ([1|0...|V], M=128):
        # the softmax denominator lands on PSUM partition 0 (canonical
        # partition_broadcast source, no relocation hop) and ctx rows sit at
        # partitions 64-127 (engine partition ranges must not cross a
        # 64-boundary from an unaligned start). Columns 1-63 are zeroed
        # once; matmul cost is N-driven so M=128 is free.
        for n in range(nqc):
            nc.vector.tensor_copy(v_t[n][:], zeros_f[:])
            nc.vector.tensor_copy(
                v_t[n][:].rearrange("p t (g c) -> p t g c", g=2)[:, :, :, 0:1],
                ones_f[:].rearrange("p (t g o) -> p t g o", g=2, o=1),
            )

        for _rep in range(repeat):
            def proj_quanta(n, xt, pools=None):
                # fill quanta at single-matmul granularity (~213ns each) so
                # the in-order PE stream interleaves finely with the
                # attention pipeline and never starves ACT. `pools` lets the
                # prologue cycle m=1 through the then-idle out-proj bank so
                # consecutive m-pieces don't serialize on one PSUM ring
                # (only safe while no out-proj quanta are pending).
                stage_box = []
                ps_box = {}
                if pools is None:
                    pools = [(psP, "proj")] * 3

                def mk_mm(m, t):
                    def f():
                        if t == 0:
                            pool, ptag = pools[m]
                            ps_box[m] = pool.tile(
                                [128, QC], F32, tag=ptag, name="ps"
                            )
                        nc.tensor.matmul(
                            ps_box[m][:],
                            w_sb[:, t, m * 128:(m + 1) * 128],
                            xt[:, t, :],
                            start=(t == 0),
                            stop=(t == 7),
                        )
                    return f

                def mk_epi(m):
                    def f():
                        ps = ps_box[m]
                        if m < 2:
                            if not stage_box:
                                stage_box.append(
                                    stpool.tile([128, 2, QC], F32R, name="stage")
                                )
                            stage = stage_box[0]
                            nc.vector.tensor_scalar_add(
                                qk_t[n][:, 0, m, :], ps[0:64, :],
                                biases[0:64, m:m + 1],
                            )
                            nc.vector.tensor_scalar_add(
                                stage[64:128, m, :], ps[64:128, :],
                                biases[64:128, m:m + 1],
                            )
                            if m == 1:
                                nc.gpsimd.dma_start(
                                    qk_t[n][:, 1, :, :], stage[64:128, :, :]
                                )
                        else:
                            vt_c = vtpool.tile([128, QC], F32R)
                            nc.vector.tensor_scalar_add(
                                vt_c[:], ps[:], biases[:, 2:3]
                            )
                            tr = psP.tile(
                                [128, 4, 128], F32R, tag="proj", name="tr"
                            )
                            for s in range(4):
                                nc.tensor.transpose(
                                    tr[:, s, :], vt_c[:, s * 128:(s + 1) * 128],
                                    ident[:],
                                )
                            nc.vector.tensor_copy(
                                v_t[n][:].rearrange(
                                    "p t (g c) -> p t g c", g=2)[:, :, :, 64:128],
                                tr[:].rearrange("p t (g c) -> p t g c", g=2),
                            )
                    return f

                out = []
                for m in range(3):
                    out.extend(mk_mm(m, t) for t in range(8))
                    out.append(mk_epi(m))
                return out

            def emit_jloop(c, fill):
                ctxA = psCA.tile([128, QC], F32, tag="ctxA")
                ctxB = psCB.tile([128, QC], F32, tag="ctxB")
                jmax = 4 * (c + 1)
                done = 0

                def emit_pv(j):
                    # PV for tile j is deferred one iteration so the in-order
                    # PE stream executes it (plus fill) while exp(j+1) runs on
                    # ACT: the QK->exp->PV latency never exposes on PE.
                    p = j - 4 * c
                    # fp32r matmul drops to 4 cycles/row below a 256-wide
                    # moving dim; widen the last diagonal tile (the extra
                    # columns are fully masked anyway)
                    off = min(max(0, p) * KT, QC - 256)
                    n_j, s_j = j // 4, j % 4
                    at = ats[j]
                    first, last = (j == 0), (j == jmax - 1)
                    nc.tensor.matmul(
                        ctxA[:, off:], v_t[n_j][:, s_j, 0:128], at[:, 0, off:],
                        start=first, stop=last,
                    )
                    nc.tensor.matmul(
                        ctxB[:, off:], v_t[n_j][:, s_j, 128:256], at[:, 1, off:],
                        start=first, stop=last,
                    )

                ats = {}
                for j in range(jmax):
                    p = j - 4 * c
                    off = min(max(0, p) * KT, QC - 256)
                    n_j, s_j = j // 4, j % 4
                    sc = psS.tile([128, 2, QC], F32, tag="sc")
                    for h in range(2):
                        nc.tensor.matmul(
                            sc[:, h, off:],
                            qk_t[n_j][:, h, 1, s_j * KT:(s_j + 1) * KT],
                            qk_t[c][:, h, 0, off:],
                            start=True, stop=True,
                        )
                    at = apool.tile([128, 2, QC], F32R)
                    nc.scalar.activation(
                        at[:, :, off:], sc[:, :, off:], AF.Exp, scale=0.125
                    )
                    if p >= 0:
                        # zero the upper-triangular wedge in place (both heads
                        # in one op; head dim has pattern step 0):
                        # keep iff (off + q_local) - k - 128*p >= 0
                        nc.gpsimd.affine_select(
                            out=at[:, :, off:], in_=at[:, :, off:],
                            pattern=[[0, 2], [1, QC - off]],
                            compare_op=AluOpType.is_ge,
                            fill=0.0, base=off - KT * p, channel_multiplier=-1,
                        )
                    ats[j] = at
                    if j > 0:
                        emit_pv(j - 1)
                        del ats[j - 1]
                    want = (len(fill) * (j + 1)) // jmax
                    while done < want:
                        fill[done]()
                        done += 1
                emit_pv(jmax - 1)
                return ctxA, ctxB

            def emit_norm_pre(c, ctxA, ctxB):
                # denominator is row 0 of each accumulator ([1|V] layout):
                # broadcast straight from partition 0, no relocation hop
                recip = npool.tile([1, 2, QC], F32, tag="recip")
                nc.vector.reciprocal(recip[0:1, 0, :], ctxA[0:1, :])
                nc.vector.reciprocal(recip[0:1, 1, :], ctxB[0:1, :])
                bc = npool.tile([128, 2, QC], F32, tag="bc")
                nc.gpsimd.partition_broadcast(bc[:], recip[0:1, :, :])
                # bufs=2: ctxn(c-2) may still be read by deferred out-proj
                # quanta while ctxn(c-1) is live and ctxn(c) is allocated.
                # Head B's normalized ctx is written DIRECTLY to ctxn[64:128]
                # (sources and dest all at partitions 64-127, aligned); only
                # head A needs a partition-relocating DMA.
                ctxn = npool.tile([128, QC], F32R, tag="ctxn", bufs=2)
                nc.vector.tensor_mul(
                    ctxn[64:128, :], ctxB[64:128, :], bc[64:128, 1, :]
                )
                ctxnA = npool.tile([128, QC], F32R, tag="ctxnA")
                nc.vector.tensor_mul(
                    ctxnA[64:128, :], ctxA[64:128, :], bc[64:128, 0, :]
                )
                nc.gpsimd.dma_start(ctxn[0:64, :], ctxnA[64:128, :])
                return ctxn

            def outproj_quanta(c, ctxn, tail=False):
                ot_box = {}

                def mk(s, half):
                    def f():
                        if half == 0:
                            ot_box[s] = opool.tile([128, D], F32, name="ot")
                        ot = ot_box[s]
                        if tail and (2 * s + half) % 2 == 1:
                            # outside the j-loop the scores ring is idle; use
                            # it as a second buffer so copies overlap matmuls
                            op = psS.tile([128, QC], F32, tag="sc", name="op")
                        else:
                            op = psQ.tile([128, QC], F32, tag="oproj", name="op")
                        nc.tensor.matmul(
                            op[:],
                            ctxn[:, s * 128:(s + 1) * 128],
                            w_o_sb[:, half * QC:(half + 1) * QC],
                            start=True, stop=True,
                        )
                        nc.vector.tensor_copy(
                            ot[:, half * QC:(half + 1) * QC], op[:]
                        )
                        if tail:
                            # start each half's writeback immediately so the
                            # final DMA drain overlaps the remaining matmuls
                            row = c * QC + s * 128
                            nc.sync.dma_start(
                                outp[row:row + 128, half * QC:(half + 1) * QC],
                                ot[:, half * QC:(half + 1) * QC],
                            )
                        elif half == 1:
                            row = c * QC + s * 128
                            nc.sync.dma_start(outp[row:row + 128, :], ot[:])
                    return f
                return [mk(s, h) for s in range(4) for h in range(2)]

            def merge(a, b):
                # proportional interleave of two quanta lists
                out, ia, ib = [], 0, 0
                while ia < len(a) or ib < len(b):
                    if ib >= len(b) or (
                        ia < len(a) and ia * len(b) <= ib * len(a)
                    ):
                        out.append(a[ia])
                        ia += 1
                    else:
                        out.append(b[ib])
                        ib += 1
                return out

            # ---- prologue (first rep reuses the preloaded x chunks)
            if _rep == 0:
                xts = dict(xts_pre)
            else:
                xts = {0: emit_xload(0)}
                if nqc > 1:
                    xts[1] = emit_xload(1)
            alt_pools = [(psP, "proj"), (psQ, "oproj"), (psP, "proj")]
            for piece in proj_quanta(0, xts[0], pools=alt_pools):
                piece()

            pending_out = []
            carry = []
            for c in range(nqc):
                fill = []
                if c + 2 < nqc:
                    def mk_load(n):
                        def f():
                            xts[n] = emit_xload(n)
                        return f
                    fill.append(mk_load(c + 2))
                projp = (
                    proj_quanta(c + 1, xts[c + 1],
                                pools=alt_pools if c == 0 else None)
                    if c + 1 < nqc else []
                )
                outq = carry + pending_out
                carry = []
                if c == nqc - 2 and outq:
                    # the last j-loop has no projection fill and starves on
                    # ACT-paced exp; defer ALL of this chunk's out-proj
                    # backlog to it (ctxn ring=2 keeps the source alive;
                    # jloop(nqc-2)'s ACT deficit is covered by proj fill)
                    carry = outq
                    outq = []
                fill += merge(projp, outq)
                ctxA, ctxB = emit_jloop(c, fill)
                ctxn = emit_norm_pre(c, ctxA, ctxB)
                pending_out = outproj_quanta(c, ctxn, tail=(c == nqc - 1))
            for piece in pending_out:
                piece()
    nc.compile()
    return nc


_PROGRAM_CACHE: dict = {}


def _get_program(S: int):
    if S not in _PROGRAM_CACHE:
        _PROGRAM_CACHE[S] = build_program(S)
    return _PROGRAM_CACHE[S]


def make_in_maps(x, w_qkv, b_qkv, w_out):
    x = np.asarray(x, dtype=np.float32)
    w_qkv = np.asarray(w_qkv, dtype=np.float32)
    b_qkv = np.asarray(b_qkv, dtype=np.float32)
    w_out = np.asarray(w_out, dtype=np.float32)
    S = x.shape[1]
    xT = np.ascontiguousarray(x.reshape(S, D).T)
    in_maps = []
    for c in range(N_CORES):
        lo, hi = 128 * c, 128 * (c + 1)
        w_shard = np.ascontiguousarray(
            np.concatenate(
                [w_qkv[:, lo:hi], w_qkv[:, D + lo:D + hi], w_qkv[:, 2 * D + lo:2 * D + hi]],
                axis=1,
            )
        )
        b_shard = np.concatenate(
            [b_qkv[lo:hi], b_qkv[D + lo:D + hi], b_qkv[2 * D + lo:2 * D + hi]]
        )
        w_o_shard = np.ascontiguousarray(w_out[lo:hi, :])
        in_maps.append(
            {"xT": xT, "w_sh": w_shard, "b_sh": b_shard, "w_o": w_o_shard}
        )
    return in_maps


def kernel(x, w_qkv, b_qkv, w_out, b_out):
    x = np.asarray(x, dtype=np.float32)
    b_out = np.asarray(b_out, dtype=np.float32)
    B, S, _ = x.shape
    in_maps = make_in_maps(x, w_qkv, b_qkv, w_out)
    nc = _get_program(S)
    res = run_bass_kernel_spmd(nc, in_maps, list(range(N_CORES))).results
    out = res[0]["outp"].copy()
    for c in range(1, N_CORES):
        out += res[c]["outp"]
    out += b_out
    return out.reshape(B, S, D)

